# revision 3
# baseline (speedup 1.0000x reference)
"""HQQLinearLoRA TRN2 kernel v3: out = x @ W + (x @ A) @ B + bias.

Data-parallel over tokens (16384) across 8 cores; per core
[2048, 4096] @ [4096, 4096] + rank-16 LoRA + bias.

v3 design:
- bf16 matmuls; the WHOLE core's x^T lives in SBUF ([128,16,32,128] bf16,
  128 KB/partition) so W is streamed from HBM exactly ONCE (~187 us);
  the 32 converted wr tiles of each n-tile are retained and reused by
  every m-walk of that n-tile.
- x^T via XBAR DMA-transpose (bf16, half-row per op). mi-major x order
  so ni=0 can start before all of x has arrived.
- ni=0 walks m in quarters (4 m-tiles, 4 psum banks) riding the x
  arrival; p1t (with a ones row for the fused K=17 LoRA+bias matmul)
  accumulates per m-tile on one borrowed bank (matmul start=True zeroes
  a whole PSUM bank on HW, so regions cannot share a bank). ni>=1 walks
  m in halves (8 banks), W of ni+1 prefetched between the halves.
- Engine map: Pool SEQ = x loads (SWDGE), Act = x f32->bf16 converts +
  psum copyouts + out-store DMAs (HWDGE), SP = W loads + x^T transposes
  (HWDGE), DVE = W converts + p1t copyouts. One dependency stream per
  sequencer: DMA triggers hold their SEQ while waiting, so cross-stream
  sharing causes priority inversion.
"""
import numpy as np
from contextlib import ExitStack

import concourse.bass as bass
import concourse.tile as tile
import concourse.mybir as mybir
from concourse import bacc
from concourse.bass_utils import run_bass_kernel_spmd

P = 128
NCORES = 8

B_DIM, S_DIM, D_DIM, R_DIM = 4, 4096, 4096, 16


def build_nc(m_core, d, r, n_tile=512, x_chunk=1024,
             ws_bufs=2, xs_bufs=3, ot_bufs=2, xb_bufs=3):
    KT = d // P                 # 32 k-tiles
    MT = m_core // P            # 16 m-tiles
    NT = d // n_tile            # 8 n-tiles
    XC = d // x_chunk           # 8 x chunks per m-tile row
    KC = x_chunk // P           # 4 k-tiles per x chunk
    QM = 4                      # m-tiles per ni=0 quarter
    NQ = MT // QM               # 4 quarters
    f32 = mybir.dt.float32
    bf16 = mybir.dt.bfloat16
    RB = r + 1                  # fused lora+bias contraction depth

    nc = bacc.Bacc(target_bir_lowering=False)
    x = nc.declare_dram_parameter("x", [m_core, d], f32, isOutput=False)
    W = nc.declare_dram_parameter("W", [d, d], f32, isOutput=False)
    bias = nc.declare_dram_parameter("bias", [d], f32, isOutput=False)
    lora_A = nc.declare_dram_parameter("lora_A", [d, r], f32, isOutput=False)
    lora_B = nc.declare_dram_parameter("lora_B", [r, d], f32, isOutput=False)
    out = nc.declare_dram_parameter("out", [m_core, d], f32, isOutput=True)

    with tile.TileContext(nc) as tc, ExitStack() as ctx:
        const = ctx.enter_context(tc.tile_pool(name="const", bufs=1))
        stg = ctx.enter_context(tc.tile_pool(name="stg", bufs=1))
        xstage = ctx.enter_context(tc.tile_pool(name="xstage", bufs=xs_bufs))
        xbfp = ctx.enter_context(tc.tile_pool(name="xbf", bufs=xb_bufs))
        wstage = ctx.enter_context(tc.tile_pool(name="wstage", bufs=ws_bufs))
        wrpool = ctx.enter_context(tc.tile_pool(name="wrpool", bufs=KT))
        outstage = ctx.enter_context(tc.tile_pool(name="outstage", bufs=ot_bufs))
        psum = ctx.enter_context(tc.tile_pool(name="psum", bufs=8, space="PSUM"))

        # ---- consts (A staged through a borrowed ws slot: same byte size)
        a_ws = wstage.tile([P, n_tile], f32, name="ws")
        a_st = a_ws[:].rearrange("p (kt r) -> p kt r", r=r)
        nc.sync.dma_start(a_st, lora_A.rearrange("(kt p) r -> p kt r", p=P))
        a_bf = const.tile([P, KT, r], bf16, name="a_bf")
        nc.vector.tensor_copy(a_bf[:], a_st)

        b17 = const.tile([RB, d], bf16, name="b17")
        for ni in range(NT):
            sl = slice(ni * n_tile, (ni + 1) * n_tile)
            bst = stg.tile([RB, n_tile], f32, name="bst")
            nc.gpsimd.dma_start(bst[0:r, :], lora_B[:, sl])
            nc.gpsimd.dma_start(bst[r:r + 1, :], bias[sl].unsqueeze(0))
            nc.vector.tensor_copy(b17[:, sl], bst[:])

        # ones row: engine writes must start at partition 0/32/64/96, so
        # memset all RB rows to 1.0 and let the chunk copyouts overwrite
        # rows 0:16; row 16 keeps the 1.0 needed for the fused bias term.
        p1t = const.tile([RB, m_core], bf16, name="p1t")
        nc.vector.memset(p1t[:], 1.0)

        # resident x^T, one tile per m-tile (keeps dep tracking per m-tile)
        xtpool = ctx.enter_context(tc.tile_pool(name="xtpool", bufs=MT))
        xtb_t = [xtpool.tile([P, KT, P], bf16, name="xt") for _ in range(MT)]

        # ---- x chain for one m-tile: loads on Pool SWDGE, f32->bf16 on the
        # Act engine into a half-row buffer, XBAR transpose per half on SP.
        def load_mtile(mt):
            nchunk = (d // 2) // x_chunk
            for h in range(2):
                xb = xbfp.tile([P, d // 2], bf16, name="xb")
                for c in range(nchunk):
                    col = h * (d // 2) + c * x_chunk
                    xs = xstage.tile([P, x_chunk], f32, name="xs")
                    nc.gpsimd.dma_start(
                        xs[:], x[mt * P:(mt + 1) * P, col:col + x_chunk])
                    nc.scalar.activation(
                        xb[:, c * x_chunk:(c + 1) * x_chunk], xs[:],
                        mybir.ActivationFunctionType.Copy)
                nc.sync.dma_start_transpose(
                    xtb_t[mt][:, h * (KT // 2):(h + 1) * (KT // 2), :], xb[:])

        # ---- matmul walk over a set of m-tiles for one n-tile
        def m_walk(ni, m0, mcnt, wrs):
            nsl = slice(ni * n_tile, (ni + 1) * n_tile)
            pss = [psum.tile([P, n_tile], f32, name="mm") for _ in range(mcnt)]
            for ki in range(KT):
                for j in range(mcnt):
                    nc.tensor.matmul(
                        pss[j][:], xtb_t[m0 + j][:, ki, :], wrs[ki][:],
                        start=(ki == 0), stop=False)
            for j in range(mcnt):
                mt = m0 + j
                nc.tensor.matmul(
                    pss[j][:], p1t[:, mt * P:(mt + 1) * P], b17[:, nsl],
                    start=False, stop=True)
                ot = outstage.tile([P, n_tile], f32, name="ot")
                nc.scalar.activation(
                    ot[:], pss[j][:], mybir.ActivationFunctionType.Copy)
                nc.scalar.dma_start(out[mt * P:(mt + 1) * P, nsl], ot[:])

        # ---- W tiles for n-tile ni: DMA + convert, tiles retained
        def load_w(ni):
            nsl = slice(ni * n_tile, (ni + 1) * n_tile)
            wrs = []
            for ki in range(KT):
                ws = wstage.tile([P, n_tile], f32, name="ws")
                nc.sync.dma_start(ws[:], W[ki * P:(ki + 1) * P, nsl])
                wr = wrpool.tile([P, n_tile], bf16, name="wr")
                nc.vector.tensor_copy(wr[:], ws[:])
                wrs.append(wr)
            return wrs

        # ---- ni = 0: quarters riding the x arrival (mi-major x order)
        wrs = load_w(0)
        for q in range(NQ):
            m0 = q * QM
            for mi in range(QM):
                load_mtile(m0 + mi)
            # p1t for the quarter's m-tiles, one borrowed bank per m-tile
            # (matmul start=True zeroes the whole PSUM bank on HW, so
            # regions of one bank cannot carry independent start flags)
            for j in range(QM):
                pt = psum.tile([P, n_tile], f32, name="mm")
                for ki in range(KT):
                    nc.tensor.matmul(
                        pt[0:r, 0:P], a_bf[:, ki, :],
                        xtb_t[m0 + j][:, ki, :],
                        start=(ki == 0), stop=(ki == KT - 1))
                nc.vector.tensor_copy(
                    p1t[0:r, (m0 + j) * P:(m0 + j + 1) * P], pt[0:r, 0:P])
            if q == NQ - 1:
                # ni=1's W DMA+converts drain while q3's k-walk frees wr
                # slots, instead of serializing after it.
                wrs_next = load_w(1)
            m_walk(0, m0, QM, wrs)

        # ---- ni >= 1: halves (8 banks), wr tiles reused across halves.
        # W for ni+1 is emitted between the halves so its DMA+convert
        # overlaps half 1 (the wr slots free as half 1's k-walk passes).
        for ni in range(1, NT):
            wrs = wrs_next
            if ni < NT - 1:
                m_walk(ni, 0, 8, wrs)
                wrs_next = load_w(ni + 1)
                m_walk(ni, 8, 8, wrs)
            else:
                m_walk(ni, 0, 8, wrs)
                for q in range(2):
                    m_walk(ni, 8 + q * QM, QM, wrs)
    nc.compile()
    return nc


_CACHE = {}


def _get_nc(key, *args, **kw):
    if key not in _CACHE:
        _CACHE[key] = build_nc(*args, **kw)
    return _CACHE[key]


def kernel(x, W, bias, lora_A, lora_B, _trace=False):
    Bb, S, D = x.shape
    R = lora_A.shape[1]
    M = Bb * S
    m_core = M // NCORES
    nc = _get_nc(("v3", m_core, D, R), m_core, D, R)

    xf = np.ascontiguousarray(x.reshape(M, D), dtype=np.float32)
    W = np.ascontiguousarray(W, dtype=np.float32)
    bias = np.ascontiguousarray(bias, dtype=np.float32)
    lora_A = np.ascontiguousarray(lora_A, dtype=np.float32)
    lora_B = np.ascontiguousarray(lora_B, dtype=np.float32)

    in_maps = []
    for c in range(NCORES):
        in_maps.append({
            "x": xf[c * m_core:(c + 1) * m_core],
            "W": W, "bias": bias, "lora_A": lora_A, "lora_B": lora_B,
        })
    res = run_bass_kernel_spmd(nc, in_maps, list(range(NCORES)), trace=_trace)
    outs = [res.results[c]["out"] for c in range(NCORES)]
    full = np.concatenate(outs, axis=0).reshape(Bb, S, D).astype(x.dtype)
    if _trace:
        return full, res
    return full


# revision 4
# speedup vs baseline: 1.0038x; 1.0038x over previous
"""HQQLinearLoRA TRN2 kernel v3: out = x @ W + (x @ A) @ B + bias.

Data-parallel over tokens (16384) across 8 cores; per core
[2048, 4096] @ [4096, 4096] + rank-16 LoRA + bias.

v3 design:
- bf16 matmuls; the WHOLE core's x^T lives in SBUF ([128,16,32,128] bf16,
  128 KB/partition) so W is streamed from HBM exactly ONCE (~187 us);
  the 32 converted wr tiles of each n-tile are retained and reused by
  every m-walk of that n-tile.
- x^T via XBAR DMA-transpose (bf16, half-row per op). mi-major x order
  so ni=0 can start before all of x has arrived.
- ni=0 walks m in quarters (4 m-tiles, 4 psum banks) riding the x
  arrival; p1t (with a ones row for the fused K=17 LoRA+bias matmul)
  accumulates per m-tile on one borrowed bank (matmul start=True zeroes
  a whole PSUM bank on HW, so regions cannot share a bank). ni>=1 walks
  m in halves (8 banks), W of ni+1 prefetched between the halves.
- Engine map: Pool SEQ = x loads (SWDGE), Act = x f32->bf16 converts +
  psum copyouts + out-store DMAs (HWDGE), SP = W loads + x^T transposes
  (HWDGE), DVE = W converts + p1t copyouts. One dependency stream per
  sequencer: DMA triggers hold their SEQ while waiting, so cross-stream
  sharing causes priority inversion.
"""
import numpy as np
from contextlib import ExitStack

import concourse.bass as bass
import concourse.tile as tile
import concourse.mybir as mybir
from concourse import bacc
from concourse.bass_utils import run_bass_kernel_spmd

P = 128
NCORES = 8

B_DIM, S_DIM, D_DIM, R_DIM = 4, 4096, 4096, 16


def build_nc(m_core, d, r, n_tile=512, x_chunk=1024,
             ws_bufs=2, xs_bufs=3, ot_bufs=2, xb_bufs=3):
    KT = d // P                 # 32 k-tiles
    MT = m_core // P            # 16 m-tiles
    NT = d // n_tile            # 8 n-tiles
    XC = d // x_chunk           # 8 x chunks per m-tile row
    KC = x_chunk // P           # 4 k-tiles per x chunk
    QM = 4                      # m-tiles per ni=0 quarter
    NQ = MT // QM               # 4 quarters
    f32 = mybir.dt.float32
    bf16 = mybir.dt.bfloat16
    RB = r + 1                  # fused lora+bias contraction depth

    nc = bacc.Bacc(target_bir_lowering=False)
    x = nc.declare_dram_parameter("x", [m_core, d], f32, isOutput=False)
    W = nc.declare_dram_parameter("W", [d, d], f32, isOutput=False)
    bias = nc.declare_dram_parameter("bias", [d], f32, isOutput=False)
    lora_A = nc.declare_dram_parameter("lora_A", [d, r], f32, isOutput=False)
    lora_B = nc.declare_dram_parameter("lora_B", [r, d], f32, isOutput=False)
    out = nc.declare_dram_parameter("out", [m_core, d], f32, isOutput=True)

    with tile.TileContext(nc) as tc, ExitStack() as ctx:
        const = ctx.enter_context(tc.tile_pool(name="const", bufs=1))
        stg = ctx.enter_context(tc.tile_pool(name="stg", bufs=1))
        xstage = ctx.enter_context(tc.tile_pool(name="xstage", bufs=xs_bufs))
        xbfp = ctx.enter_context(tc.tile_pool(name="xbf", bufs=xb_bufs))
        wstage = ctx.enter_context(tc.tile_pool(name="wstage", bufs=ws_bufs))
        wrpool = ctx.enter_context(tc.tile_pool(name="wrpool", bufs=KT))
        outstage = ctx.enter_context(tc.tile_pool(name="outstage", bufs=ot_bufs))
        psum = ctx.enter_context(tc.tile_pool(name="psum", bufs=8, space="PSUM"))

        # ---- consts (A staged through a borrowed ws slot: same byte size)
        a_ws = wstage.tile([P, n_tile], f32, name="ws")
        a_st = a_ws[:].rearrange("p (kt r) -> p kt r", r=r)
        nc.sync.dma_start(a_st, lora_A.rearrange("(kt p) r -> p kt r", p=P))
        a_bf = const.tile([P, KT, r], bf16, name="a_bf")
        nc.vector.tensor_copy(a_bf[:], a_st)

        b17 = const.tile([RB, d], bf16, name="b17")
        for ni in range(NT):
            sl = slice(ni * n_tile, (ni + 1) * n_tile)
            bst = stg.tile([RB, n_tile], f32, name="bst")
            nc.gpsimd.dma_start(bst[0:r, :], lora_B[:, sl])
            nc.gpsimd.dma_start(bst[r:r + 1, :], bias[sl].unsqueeze(0))
            nc.vector.tensor_copy(b17[:, sl], bst[:])

        # ones row: engine writes must start at partition 0/32/64/96, so
        # memset all RB rows to 1.0 and let the chunk copyouts overwrite
        # rows 0:16; row 16 keeps the 1.0 needed for the fused bias term.
        p1t = const.tile([RB, m_core], bf16, name="p1t")
        nc.vector.memset(p1t[:], 1.0)

        # resident x^T, one tile per m-tile (keeps dep tracking per m-tile)
        xtpool = ctx.enter_context(tc.tile_pool(name="xtpool", bufs=MT))
        xtb_t = [xtpool.tile([P, KT, P], bf16, name="xt") for _ in range(MT)]

        # ---- x chain for one m-tile: loads on Pool SWDGE, f32->bf16 on the
        # Act engine into a half-row buffer, XBAR transpose per half on SP.
        def load_mtile(mt):
            nchunk = (d // 2) // x_chunk
            for h in range(2):
                xb = xbfp.tile([P, d // 2], bf16, name="xb")
                for c in range(nchunk):
                    col = h * (d // 2) + c * x_chunk
                    xs = xstage.tile([P, x_chunk], f32, name="xs")
                    nc.gpsimd.dma_start(
                        xs[:], x[mt * P:(mt + 1) * P, col:col + x_chunk])
                    nc.scalar.activation(
                        xb[:, c * x_chunk:(c + 1) * x_chunk], xs[:],
                        mybir.ActivationFunctionType.Copy)
                nc.sync.dma_start_transpose(
                    xtb_t[mt][:, h * (KT // 2):(h + 1) * (KT // 2), :], xb[:])

        # ---- matmul walk over a set of m-tiles for one n-tile
        def m_walk(ni, m0, mcnt, wrs):
            nsl = slice(ni * n_tile, (ni + 1) * n_tile)
            pss = [psum.tile([P, n_tile], f32, name="mm") for _ in range(mcnt)]
            for ki in range(KT):
                for j in range(mcnt):
                    nc.tensor.matmul(
                        pss[j][:], xtb_t[m0 + j][:, ki, :], wrs[ki][:],
                        start=(ki == 0), stop=False)
            for j in range(mcnt):
                mt = m0 + j
                nc.tensor.matmul(
                    pss[j][:], p1t[:, mt * P:(mt + 1) * P], b17[:, nsl],
                    start=False, stop=True)
                ot = outstage.tile([P, n_tile], f32, name="ot")
                nc.scalar.activation(
                    ot[:], pss[j][:], mybir.ActivationFunctionType.Copy)
                nc.scalar.dma_start(out[mt * P:(mt + 1) * P, nsl], ot[:])

        # ---- W tiles for n-tile ni: DMA + convert, tiles retained
        def load_w(ni):
            nsl = slice(ni * n_tile, (ni + 1) * n_tile)
            wrs = []
            for ki in range(KT):
                ws = wstage.tile([P, n_tile], f32, name="ws")
                nc.sync.dma_start(ws[:], W[ki * P:(ki + 1) * P, nsl])
                wr = wrpool.tile([P, n_tile], bf16, name="wr")
                nc.vector.tensor_copy(wr[:], ws[:])
                wrs.append(wr)
            return wrs

        # ---- ni = 0: quarters riding the x arrival (mi-major x order)
        wrs = load_w(0)
        for q in range(NQ):
            m0 = q * QM
            for mi in range(QM):
                load_mtile(m0 + mi)
            # p1t for the quarter's m-tiles, one borrowed bank per m-tile
            # (matmul start=True zeroes the whole PSUM bank on HW, so
            # regions of one bank cannot carry independent start flags)
            for j in range(QM):
                pt = psum.tile([P, n_tile], f32, name="mm")
                for ki in range(KT):
                    nc.tensor.matmul(
                        pt[0:r, 0:P], a_bf[:, ki, :],
                        xtb_t[m0 + j][:, ki, :],
                        start=(ki == 0), stop=(ki == KT - 1))
                nc.vector.tensor_copy(
                    p1t[0:r, (m0 + j) * P:(m0 + j + 1) * P], pt[0:r, 0:P])
            if q == NQ - 1:
                # ni=1's W DMA+converts drain while q3's k-walk frees wr
                # slots, instead of serializing after it.
                wrs_next = load_w(1)
            m_walk(0, m0, QM, wrs)

        # ---- ni >= 1: halves (8 banks), wr tiles reused across halves.
        # W for ni+1 is emitted between the halves so its DMA+convert
        # overlaps half 1 (the wr slots free as half 1's k-walk passes).
        for ni in range(1, NT):
            wrs = wrs_next
            if ni < NT - 1:
                m_walk(ni, 0, 8, wrs)
                wrs_next = load_w(ni + 1)
                m_walk(ni, 8, 8, wrs)
            else:
                # taper the final walks so the drain pipeline (copyout +
                # out DMA) empties during remaining matmuls, not after
                m_walk(ni, 0, 8, wrs)
                m_walk(ni, 8, 4, wrs)
                m_walk(ni, 12, 2, wrs)
                m_walk(ni, 14, 2, wrs)
    nc.compile()
    return nc


_CACHE = {}


def _get_nc(key, *args, **kw):
    if key not in _CACHE:
        _CACHE[key] = build_nc(*args, **kw)
    return _CACHE[key]


def kernel(x, W, bias, lora_A, lora_B, _trace=False):
    Bb, S, D = x.shape
    R = lora_A.shape[1]
    M = Bb * S
    m_core = M // NCORES
    nc = _get_nc(("v3", m_core, D, R), m_core, D, R)

    xf = np.ascontiguousarray(x.reshape(M, D), dtype=np.float32)
    W = np.ascontiguousarray(W, dtype=np.float32)
    bias = np.ascontiguousarray(bias, dtype=np.float32)
    lora_A = np.ascontiguousarray(lora_A, dtype=np.float32)
    lora_B = np.ascontiguousarray(lora_B, dtype=np.float32)

    in_maps = []
    for c in range(NCORES):
        in_maps.append({
            "x": xf[c * m_core:(c + 1) * m_core],
            "W": W, "bias": bias, "lora_A": lora_A, "lora_B": lora_B,
        })
    res = run_bass_kernel_spmd(nc, in_maps, list(range(NCORES)), trace=_trace)
    outs = [res.results[c]["out"] for c in range(NCORES)]
    full = np.concatenate(outs, axis=0).reshape(Bb, S, D).astype(x.dtype)
    if _trace:
        return full, res
    return full


# revision 5
# speedup vs baseline: 1.0067x; 1.0029x over previous
"""HQQLinearLoRA TRN2 kernel v3: out = x @ W + (x @ A) @ B + bias.

Data-parallel over tokens (16384) across 8 cores; per core
[2048, 4096] @ [4096, 4096] + rank-16 LoRA + bias.

v3 design:
- bf16 matmuls; the WHOLE core's x^T lives in SBUF ([128,16,32,128] bf16,
  128 KB/partition) so W is streamed from HBM exactly ONCE (~187 us);
  the 32 converted wr tiles of each n-tile are retained and reused by
  every m-walk of that n-tile.
- x^T via XBAR DMA-transpose (bf16, half-row per op). mi-major x order
  so ni=0 can start before all of x has arrived.
- ni=0 walks m in quarters (4 m-tiles, 4 psum banks) riding the x
  arrival; p1t (with a ones row for the fused K=17 LoRA+bias matmul)
  accumulates per m-tile on one borrowed bank (matmul start=True zeroes
  a whole PSUM bank on HW, so regions cannot share a bank). ni>=1 walks
  m in halves (8 banks), W of ni+1 prefetched between the halves.
- Engine map: Pool SEQ = x loads (SWDGE), Act = x f32->bf16 converts +
  psum copyouts + out-store DMAs (HWDGE), SP = W loads + x^T transposes
  (HWDGE), DVE = W converts + p1t copyouts. One dependency stream per
  sequencer: DMA triggers hold their SEQ while waiting, so cross-stream
  sharing causes priority inversion.
"""
import numpy as np
from contextlib import ExitStack

import concourse.bass as bass
import concourse.tile as tile
import concourse.mybir as mybir
from concourse import bacc
from concourse.bass_utils import run_bass_kernel_spmd

P = 128
NCORES = 8

B_DIM, S_DIM, D_DIM, R_DIM = 4, 4096, 4096, 16


def build_nc(m_core, d, r, n_tile=512, x_chunk=1024,
             ws_bufs=2, xs_bufs=4, ot_bufs=2, xb_bufs=2):
    KT = d // P                 # 32 k-tiles
    MT = m_core // P            # 16 m-tiles
    NT = d // n_tile            # 8 n-tiles
    XC = d // x_chunk           # 8 x chunks per m-tile row
    KC = x_chunk // P           # 4 k-tiles per x chunk
    QM = 4                      # m-tiles per ni=0 quarter
    NQ = MT // QM               # 4 quarters
    f32 = mybir.dt.float32
    bf16 = mybir.dt.bfloat16
    RB = r + 1                  # fused lora+bias contraction depth

    nc = bacc.Bacc(target_bir_lowering=False)
    x = nc.declare_dram_parameter("x", [m_core, d], f32, isOutput=False)
    W = nc.declare_dram_parameter("W", [d, d], f32, isOutput=False)
    bias = nc.declare_dram_parameter("bias", [d], f32, isOutput=False)
    lora_A = nc.declare_dram_parameter("lora_A", [d, r], f32, isOutput=False)
    lora_B = nc.declare_dram_parameter("lora_B", [r, d], f32, isOutput=False)
    out = nc.declare_dram_parameter("out", [m_core, d], f32, isOutput=True)

    with tile.TileContext(nc) as tc, ExitStack() as ctx:
        const = ctx.enter_context(tc.tile_pool(name="const", bufs=1))
        stg = ctx.enter_context(tc.tile_pool(name="stg", bufs=1))
        xstage = ctx.enter_context(tc.tile_pool(name="xstage", bufs=xs_bufs))
        xbfp = ctx.enter_context(tc.tile_pool(name="xbf", bufs=xb_bufs))
        wstage = ctx.enter_context(tc.tile_pool(name="wstage", bufs=ws_bufs))
        wrpool = ctx.enter_context(tc.tile_pool(name="wrpool", bufs=KT))
        outstage = ctx.enter_context(tc.tile_pool(name="outstage", bufs=ot_bufs))
        psum = ctx.enter_context(tc.tile_pool(name="psum", bufs=8, space="PSUM"))

        # ---- consts (A staged through a borrowed ws slot: same byte size)
        a_ws = wstage.tile([P, n_tile], f32, name="ws")
        a_st = a_ws[:].rearrange("p (kt r) -> p kt r", r=r)
        nc.sync.dma_start(a_st, lora_A.rearrange("(kt p) r -> p kt r", p=P))
        a_bf = const.tile([P, KT, r], bf16, name="a_bf")
        nc.vector.tensor_copy(a_bf[:], a_st)

        b17 = const.tile([RB, d], bf16, name="b17")
        for ni in range(NT):
            sl = slice(ni * n_tile, (ni + 1) * n_tile)
            bst = stg.tile([RB, n_tile], f32, name="bst")
            nc.gpsimd.dma_start(bst[0:r, :], lora_B[:, sl])
            nc.gpsimd.dma_start(bst[r:r + 1, :], bias[sl].unsqueeze(0))
            nc.vector.tensor_copy(b17[:, sl], bst[:])

        # ones row: engine writes must start at partition 0/32/64/96, so
        # memset all RB rows to 1.0 and let the chunk copyouts overwrite
        # rows 0:16; row 16 keeps the 1.0 needed for the fused bias term.
        p1t = const.tile([RB, m_core], bf16, name="p1t")
        nc.vector.memset(p1t[:], 1.0)

        # resident x^T, one tile per m-tile (keeps dep tracking per m-tile)
        xtpool = ctx.enter_context(tc.tile_pool(name="xtpool", bufs=MT))
        xtb_t = [xtpool.tile([P, KT, P], bf16, name="xt") for _ in range(MT)]

        # ---- x chain for one m-tile: loads on Pool SWDGE, f32->bf16 on the
        # Act engine into a half-row buffer, XBAR transpose per half on SP.
        def load_mtile(mt):
            nchunk = (d // 2) // x_chunk
            for h in range(2):
                xb = xbfp.tile([P, d // 2], bf16, name="xb")
                for c in range(nchunk):
                    col = h * (d // 2) + c * x_chunk
                    xs = xstage.tile([P, x_chunk], f32, name="xs")
                    nc.gpsimd.dma_start(
                        xs[:], x[mt * P:(mt + 1) * P, col:col + x_chunk])
                    nc.scalar.activation(
                        xb[:, c * x_chunk:(c + 1) * x_chunk], xs[:],
                        mybir.ActivationFunctionType.Copy)
                nc.sync.dma_start_transpose(
                    xtb_t[mt][:, h * (KT // 2):(h + 1) * (KT // 2), :], xb[:])

        # ---- matmul walk over a set of m-tiles for one n-tile
        def m_walk(ni, m0, mcnt, wrs):
            nsl = slice(ni * n_tile, (ni + 1) * n_tile)
            pss = [psum.tile([P, n_tile], f32, name="mm") for _ in range(mcnt)]
            for ki in range(KT):
                for j in range(mcnt):
                    nc.tensor.matmul(
                        pss[j][:], xtb_t[m0 + j][:, ki, :], wrs[ki][:],
                        start=(ki == 0), stop=False)
            for j in range(mcnt):
                mt = m0 + j
                nc.tensor.matmul(
                    pss[j][:], p1t[:, mt * P:(mt + 1) * P], b17[:, nsl],
                    start=False, stop=True)
                ot = outstage.tile([P, n_tile], f32, name="ot")
                nc.scalar.activation(
                    ot[:], pss[j][:], mybir.ActivationFunctionType.Copy)
                nc.scalar.dma_start(out[mt * P:(mt + 1) * P, nsl], ot[:])

        # ---- W tiles for n-tile ni: DMA + convert, tiles retained
        def load_w(ni):
            nsl = slice(ni * n_tile, (ni + 1) * n_tile)
            wrs = []
            for ki in range(KT):
                ws = wstage.tile([P, n_tile], f32, name="ws")
                nc.sync.dma_start(ws[:], W[ki * P:(ki + 1) * P, nsl])
                wr = wrpool.tile([P, n_tile], bf16, name="wr")
                nc.vector.tensor_copy(wr[:], ws[:])
                wrs.append(wr)
            return wrs

        # ---- ni = 0: quarters riding the x arrival (mi-major x order)
        wrs = load_w(0)
        for q in range(NQ):
            m0 = q * QM
            for mi in range(QM):
                load_mtile(m0 + mi)
            # p1t for the quarter's m-tiles, one borrowed bank per m-tile
            # (matmul start=True zeroes the whole PSUM bank on HW, so
            # regions of one bank cannot carry independent start flags)
            for j in range(QM):
                pt = psum.tile([P, n_tile], f32, name="mm")
                for ki in range(KT):
                    nc.tensor.matmul(
                        pt[0:r, 0:P], a_bf[:, ki, :],
                        xtb_t[m0 + j][:, ki, :],
                        start=(ki == 0), stop=(ki == KT - 1))
                nc.vector.tensor_copy(
                    p1t[0:r, (m0 + j) * P:(m0 + j + 1) * P], pt[0:r, 0:P])
            if q == NQ - 1:
                # ni=1's W DMA+converts drain while q3's k-walk frees wr
                # slots, instead of serializing after it.
                wrs_next = load_w(1)
            m_walk(0, m0, QM, wrs)

        # ---- ni >= 1: halves (8 banks), wr tiles reused across halves.
        # W for ni+1 is emitted between the halves so its DMA+convert
        # overlaps half 1 (the wr slots free as half 1's k-walk passes).
        for ni in range(1, NT):
            wrs = wrs_next
            if ni < NT - 1:
                m_walk(ni, 0, 8, wrs)
                wrs_next = load_w(ni + 1)
                m_walk(ni, 8, 8, wrs)
            else:
                # taper the final walks so the drain pipeline (copyout +
                # out DMA) empties during remaining matmuls, not after
                m_walk(ni, 0, 8, wrs)
                m_walk(ni, 8, 4, wrs)
                m_walk(ni, 12, 2, wrs)
                m_walk(ni, 14, 2, wrs)
    nc.compile()
    return nc


_CACHE = {}


def _get_nc(key, *args, **kw):
    if key not in _CACHE:
        _CACHE[key] = build_nc(*args, **kw)
    return _CACHE[key]


def kernel(x, W, bias, lora_A, lora_B, _trace=False):
    Bb, S, D = x.shape
    R = lora_A.shape[1]
    M = Bb * S
    m_core = M // NCORES
    nc = _get_nc(("v3", m_core, D, R), m_core, D, R)

    xf = np.ascontiguousarray(x.reshape(M, D), dtype=np.float32)
    W = np.ascontiguousarray(W, dtype=np.float32)
    bias = np.ascontiguousarray(bias, dtype=np.float32)
    lora_A = np.ascontiguousarray(lora_A, dtype=np.float32)
    lora_B = np.ascontiguousarray(lora_B, dtype=np.float32)

    in_maps = []
    for c in range(NCORES):
        in_maps.append({
            "x": xf[c * m_core:(c + 1) * m_core],
            "W": W, "bias": bias, "lora_A": lora_A, "lora_B": lora_B,
        })
    res = run_bass_kernel_spmd(nc, in_maps, list(range(NCORES)), trace=_trace)
    outs = [res.results[c]["out"] for c in range(NCORES)]
    full = np.concatenate(outs, axis=0).reshape(Bb, S, D).astype(x.dtype)
    if _trace:
        return full, res
    return full


# revision 6
# speedup vs baseline: 1.0185x; 1.0117x over previous
"""HQQLinearLoRA TRN2 kernel v3: out = x @ W + (x @ A) @ B + bias.

Data-parallel over tokens (16384) across 8 cores; per core
[2048, 4096] @ [4096, 4096] + rank-16 LoRA + bias.

v3 design:
- bf16 matmuls; the WHOLE core's x^T lives in SBUF ([128,16,32,128] bf16,
  128 KB/partition) so W is streamed from HBM exactly ONCE (~187 us);
  the 32 converted wr tiles of each n-tile are retained and reused by
  every m-walk of that n-tile.
- x^T via XBAR DMA-transpose (bf16, half-row per op). mi-major x order
  so ni=0 can start before all of x has arrived.
- ni=0 walks m in quarters (4 m-tiles, 4 psum banks) riding the x
  arrival; p1t (with a ones row for the fused K=17 LoRA+bias matmul)
  accumulates per m-tile on one borrowed bank (matmul start=True zeroes
  a whole PSUM bank on HW, so regions cannot share a bank). ni>=1 walks
  m in halves (8 banks), W of ni+1 prefetched between the halves.
- Engine map: Pool SEQ = x loads (SWDGE), Act = x f32->bf16 converts +
  psum copyouts + out-store DMAs (HWDGE), SP = W loads + x^T transposes
  (HWDGE), DVE = W converts + p1t copyouts. One dependency stream per
  sequencer: DMA triggers hold their SEQ while waiting, so cross-stream
  sharing causes priority inversion.
"""
import numpy as np
from contextlib import ExitStack

import concourse.bass as bass
import concourse.tile as tile
import concourse.mybir as mybir
from concourse import bacc
from concourse.bass_utils import run_bass_kernel_spmd

P = 128
NCORES = 8

B_DIM, S_DIM, D_DIM, R_DIM = 4, 4096, 4096, 16


def build_nc(m_core, d, r, n_tile=512, x_chunk=1024,
             ws_bufs=2, xs_bufs=4, ot_bufs=2, xb_bufs=2):
    KT = d // P                 # 32 k-tiles
    MT = m_core // P            # 16 m-tiles
    NT = d // n_tile            # 8 n-tiles
    XC = d // x_chunk           # 8 x chunks per m-tile row
    KC = x_chunk // P           # 4 k-tiles per x chunk
    QM = 2                      # m-tiles per ni=0 group (pairs track x arrival)
    NQ = MT // QM               # 4 quarters
    f32 = mybir.dt.float32
    bf16 = mybir.dt.bfloat16
    RB = r + 1                  # fused lora+bias contraction depth

    nc = bacc.Bacc(target_bir_lowering=False)
    x = nc.declare_dram_parameter("x", [m_core, d], f32, isOutput=False)
    W = nc.declare_dram_parameter("W", [d, d], f32, isOutput=False)
    bias = nc.declare_dram_parameter("bias", [d], f32, isOutput=False)
    lora_A = nc.declare_dram_parameter("lora_A", [d, r], f32, isOutput=False)
    lora_B = nc.declare_dram_parameter("lora_B", [r, d], f32, isOutput=False)
    out = nc.declare_dram_parameter("out", [m_core, d], f32, isOutput=True)

    with tile.TileContext(nc) as tc, ExitStack() as ctx:
        const = ctx.enter_context(tc.tile_pool(name="const", bufs=1))
        stg = ctx.enter_context(tc.tile_pool(name="stg", bufs=1))
        xstage = ctx.enter_context(tc.tile_pool(name="xstage", bufs=xs_bufs))
        xbfp = ctx.enter_context(tc.tile_pool(name="xbf", bufs=xb_bufs))
        wstage = ctx.enter_context(tc.tile_pool(name="wstage", bufs=ws_bufs))
        wrpool = ctx.enter_context(tc.tile_pool(name="wrpool", bufs=KT))
        outstage = ctx.enter_context(tc.tile_pool(name="outstage", bufs=ot_bufs))
        psum = ctx.enter_context(tc.tile_pool(name="psum", bufs=8, space="PSUM"))

        # ---- consts (A staged through a borrowed ws slot: same byte size)
        a_ws = wstage.tile([P, n_tile], f32, name="ws")
        a_st = a_ws[:].rearrange("p (kt r) -> p kt r", r=r)
        nc.sync.dma_start(a_st, lora_A.rearrange("(kt p) r -> p kt r", p=P))
        a_bf = const.tile([P, KT, r], bf16, name="a_bf")
        nc.vector.tensor_copy(a_bf[:], a_st)

        b17 = const.tile([RB, d], bf16, name="b17")
        for ni in range(NT):
            sl = slice(ni * n_tile, (ni + 1) * n_tile)
            bst = stg.tile([RB, n_tile], f32, name="bst")
            nc.gpsimd.dma_start(bst[0:r, :], lora_B[:, sl])
            nc.gpsimd.dma_start(bst[r:r + 1, :], bias[sl].unsqueeze(0))
            nc.vector.tensor_copy(b17[:, sl], bst[:])

        # ones row: engine writes must start at partition 0/32/64/96, so
        # memset all RB rows to 1.0 and let the chunk copyouts overwrite
        # rows 0:16; row 16 keeps the 1.0 needed for the fused bias term.
        p1t = const.tile([RB, m_core], bf16, name="p1t")
        nc.vector.memset(p1t[:], 1.0)

        # resident x^T, one tile per m-tile (keeps dep tracking per m-tile)
        xtpool = ctx.enter_context(tc.tile_pool(name="xtpool", bufs=MT))
        xtb_t = [xtpool.tile([P, KT, P], bf16, name="xt") for _ in range(MT)]

        # ---- x chain for one m-tile: loads on Pool SWDGE, f32->bf16 on the
        # Act engine into a half-row buffer, XBAR transpose per half on SP.
        def load_mtile(mt):
            nchunk = (d // 2) // x_chunk
            for h in range(2):
                xb = xbfp.tile([P, d // 2], bf16, name="xb")
                for c in range(nchunk):
                    col = h * (d // 2) + c * x_chunk
                    xs = xstage.tile([P, x_chunk], f32, name="xs")
                    nc.gpsimd.dma_start(
                        xs[:], x[mt * P:(mt + 1) * P, col:col + x_chunk])
                    nc.scalar.activation(
                        xb[:, c * x_chunk:(c + 1) * x_chunk], xs[:],
                        mybir.ActivationFunctionType.Copy)
                nc.sync.dma_start_transpose(
                    xtb_t[mt][:, h * (KT // 2):(h + 1) * (KT // 2), :], xb[:])

        # ---- matmul walk over a set of m-tiles for one n-tile
        def m_walk(ni, m0, mcnt, wrs):
            nsl = slice(ni * n_tile, (ni + 1) * n_tile)
            pss = [psum.tile([P, n_tile], f32, name="mm") for _ in range(mcnt)]
            for ki in range(KT):
                for j in range(mcnt):
                    nc.tensor.matmul(
                        pss[j][:], xtb_t[m0 + j][:, ki, :], wrs[ki][:],
                        start=(ki == 0), stop=False)
            for j in range(mcnt):
                mt = m0 + j
                nc.tensor.matmul(
                    pss[j][:], p1t[:, mt * P:(mt + 1) * P], b17[:, nsl],
                    start=False, stop=True)
                ot = outstage.tile([P, n_tile], f32, name="ot")
                nc.scalar.activation(
                    ot[:], pss[j][:], mybir.ActivationFunctionType.Copy)
                nc.scalar.dma_start(out[mt * P:(mt + 1) * P, nsl], ot[:])

        # ---- W tiles for n-tile ni: DMA + convert, tiles retained
        def load_w(ni):
            nsl = slice(ni * n_tile, (ni + 1) * n_tile)
            wrs = []
            for ki in range(KT):
                ws = wstage.tile([P, n_tile], f32, name="ws")
                nc.sync.dma_start(ws[:], W[ki * P:(ki + 1) * P, nsl])
                wr = wrpool.tile([P, n_tile], bf16, name="wr")
                nc.vector.tensor_copy(wr[:], ws[:])
                wrs.append(wr)
            return wrs

        # ---- ni = 0: quarters riding the x arrival (mi-major x order)
        wrs = load_w(0)
        for q in range(NQ):
            m0 = q * QM
            for mi in range(QM):
                load_mtile(m0 + mi)
            # p1t for the quarter's m-tiles, one borrowed bank per m-tile
            # (matmul start=True zeroes the whole PSUM bank on HW, so
            # regions of one bank cannot carry independent start flags)
            for j in range(QM):
                pt = psum.tile([P, n_tile], f32, name="mm")
                for ki in range(KT):
                    nc.tensor.matmul(
                        pt[0:r, 0:P], a_bf[:, ki, :],
                        xtb_t[m0 + j][:, ki, :],
                        start=(ki == 0), stop=(ki == KT - 1))
                nc.vector.tensor_copy(
                    p1t[0:r, (m0 + j) * P:(m0 + j + 1) * P], pt[0:r, 0:P])
            if q == NQ - 1:
                # ni=1's W DMA+converts drain while q3's k-walk frees wr
                # slots, instead of serializing after it.
                wrs_next = load_w(1)
            m_walk(0, m0, QM, wrs)

        # ---- ni >= 1: halves (8 banks), wr tiles reused across halves.
        # W for ni+1 is emitted between the halves so its DMA+convert
        # overlaps half 1 (the wr slots free as half 1's k-walk passes).
        for ni in range(1, NT):
            wrs = wrs_next
            if ni < NT - 1:
                m_walk(ni, 0, 8, wrs)
                wrs_next = load_w(ni + 1)
                m_walk(ni, 8, 8, wrs)
            else:
                # taper the final walks so the drain pipeline (copyout +
                # out DMA) empties during remaining matmuls, not after
                m_walk(ni, 0, 8, wrs)
                m_walk(ni, 8, 4, wrs)
                m_walk(ni, 12, 2, wrs)
                m_walk(ni, 14, 2, wrs)
    nc.compile()
    return nc


_CACHE = {}


def _get_nc(key, *args, **kw):
    if key not in _CACHE:
        _CACHE[key] = build_nc(*args, **kw)
    return _CACHE[key]


def kernel(x, W, bias, lora_A, lora_B, _trace=False):
    Bb, S, D = x.shape
    R = lora_A.shape[1]
    M = Bb * S
    m_core = M // NCORES
    nc = _get_nc(("v3", m_core, D, R), m_core, D, R)

    xf = np.ascontiguousarray(x.reshape(M, D), dtype=np.float32)
    W = np.ascontiguousarray(W, dtype=np.float32)
    bias = np.ascontiguousarray(bias, dtype=np.float32)
    lora_A = np.ascontiguousarray(lora_A, dtype=np.float32)
    lora_B = np.ascontiguousarray(lora_B, dtype=np.float32)

    in_maps = []
    for c in range(NCORES):
        in_maps.append({
            "x": xf[c * m_core:(c + 1) * m_core],
            "W": W, "bias": bias, "lora_A": lora_A, "lora_B": lora_B,
        })
    res = run_bass_kernel_spmd(nc, in_maps, list(range(NCORES)), trace=_trace)
    outs = [res.results[c]["out"] for c in range(NCORES)]
    full = np.concatenate(outs, axis=0).reshape(Bb, S, D).astype(x.dtype)
    if _trace:
        return full, res
    return full


# revision 7
# speedup vs baseline: 1.0300x; 1.0113x over previous
"""HQQLinearLoRA TRN2 kernel v3: out = x @ W + (x @ A) @ B + bias.

Data-parallel over tokens (16384) across 8 cores; per core
[2048, 4096] @ [4096, 4096] + rank-16 LoRA + bias.

v3 design:
- bf16 matmuls; the WHOLE core's x^T lives in SBUF ([128,16,32,128] bf16,
  128 KB/partition) so W is streamed from HBM exactly ONCE (~187 us);
  the 32 converted wr tiles of each n-tile are retained and reused by
  every m-walk of that n-tile.
- x^T via XBAR DMA-transpose (bf16, half-row per op). mi-major x order
  so ni=0 can start before all of x has arrived.
- ni=0 walks m in quarters (4 m-tiles, 4 psum banks) riding the x
  arrival; p1t (with a ones row for the fused K=17 LoRA+bias matmul)
  accumulates per m-tile on one borrowed bank (matmul start=True zeroes
  a whole PSUM bank on HW, so regions cannot share a bank). ni>=1 walks
  m in halves (8 banks), W of ni+1 prefetched between the halves.
- Engine map: Pool SEQ = x loads (SWDGE), Act = x f32->bf16 converts +
  psum copyouts + out-store DMAs (HWDGE), SP = W loads + x^T transposes
  (HWDGE), DVE = W converts + p1t copyouts. One dependency stream per
  sequencer: DMA triggers hold their SEQ while waiting, so cross-stream
  sharing causes priority inversion.
"""
import numpy as np
from contextlib import ExitStack

import concourse.bass as bass
import concourse.tile as tile
import concourse.mybir as mybir
from concourse import bacc
from concourse.bass_utils import run_bass_kernel_spmd

P = 128
NCORES = 8

B_DIM, S_DIM, D_DIM, R_DIM = 4, 4096, 4096, 16


def build_nc(m_core, d, r, n_tile=512, x_chunk=1024,
             ws_bufs=2, xs_bufs=4, ot_bufs=2, xb_bufs=2):
    KT = d // P                 # 32 k-tiles
    MT = m_core // P            # 16 m-tiles
    NT = d // n_tile            # 8 n-tiles
    XC = d // x_chunk           # 8 x chunks per m-tile row
    KC = x_chunk // P           # 4 k-tiles per x chunk
    QM = 1                      # m-tiles per ni=0 group (tracks x arrival)
    NQ = MT // QM               # 4 quarters
    f32 = mybir.dt.float32
    bf16 = mybir.dt.bfloat16
    RB = r + 1                  # fused lora+bias contraction depth

    nc = bacc.Bacc(target_bir_lowering=False)
    x = nc.declare_dram_parameter("x", [m_core, d], f32, isOutput=False)
    W = nc.declare_dram_parameter("W", [d, d], f32, isOutput=False)
    bias = nc.declare_dram_parameter("bias", [d], f32, isOutput=False)
    lora_A = nc.declare_dram_parameter("lora_A", [d, r], f32, isOutput=False)
    lora_B = nc.declare_dram_parameter("lora_B", [r, d], f32, isOutput=False)
    out = nc.declare_dram_parameter("out", [m_core, d], f32, isOutput=True)

    with tile.TileContext(nc) as tc, ExitStack() as ctx:
        const = ctx.enter_context(tc.tile_pool(name="const", bufs=1))
        stg = ctx.enter_context(tc.tile_pool(name="stg", bufs=1))
        xstage = ctx.enter_context(tc.tile_pool(name="xstage", bufs=xs_bufs))
        xbfp = ctx.enter_context(tc.tile_pool(name="xbf", bufs=xb_bufs))
        wstage = ctx.enter_context(tc.tile_pool(name="wstage", bufs=ws_bufs))
        wrpool = ctx.enter_context(tc.tile_pool(name="wrpool", bufs=KT))
        outstage = ctx.enter_context(tc.tile_pool(name="outstage", bufs=ot_bufs))
        psum = ctx.enter_context(tc.tile_pool(name="psum", bufs=8, space="PSUM"))

        # ---- consts (A staged through a borrowed ws slot: same byte size)
        a_ws = wstage.tile([P, n_tile], f32, name="ws")
        a_st = a_ws[:].rearrange("p (kt r) -> p kt r", r=r)
        nc.sync.dma_start(a_st, lora_A.rearrange("(kt p) r -> p kt r", p=P))
        a_bf = const.tile([P, KT, r], bf16, name="a_bf")
        nc.vector.tensor_copy(a_bf[:], a_st)

        b17 = const.tile([RB, d], bf16, name="b17")
        for ni in range(NT):
            sl = slice(ni * n_tile, (ni + 1) * n_tile)
            bst = stg.tile([RB, n_tile], f32, name="bst")
            nc.gpsimd.dma_start(bst[0:r, :], lora_B[:, sl])
            nc.gpsimd.dma_start(bst[r:r + 1, :], bias[sl].unsqueeze(0))
            nc.vector.tensor_copy(b17[:, sl], bst[:])

        # ones row: engine writes must start at partition 0/32/64/96, so
        # memset all RB rows to 1.0 and let the chunk copyouts overwrite
        # rows 0:16; row 16 keeps the 1.0 needed for the fused bias term.
        p1t = const.tile([RB, m_core], bf16, name="p1t")
        nc.vector.memset(p1t[:], 1.0)

        # resident x^T, one tile per m-tile (keeps dep tracking per m-tile)
        xtpool = ctx.enter_context(tc.tile_pool(name="xtpool", bufs=MT))
        xtb_t = [xtpool.tile([P, KT, P], bf16, name="xt") for _ in range(MT)]

        # ---- x chain for one m-tile: loads on Pool SWDGE, f32->bf16 on the
        # Act engine into a half-row buffer, XBAR transpose per half on SP.
        def load_mtile(mt):
            nchunk = (d // 2) // x_chunk
            for h in range(2):
                xb = xbfp.tile([P, d // 2], bf16, name="xb")
                for c in range(nchunk):
                    col = h * (d // 2) + c * x_chunk
                    xs = xstage.tile([P, x_chunk], f32, name="xs")
                    nc.gpsimd.dma_start(
                        xs[:], x[mt * P:(mt + 1) * P, col:col + x_chunk])
                    nc.scalar.activation(
                        xb[:, c * x_chunk:(c + 1) * x_chunk], xs[:],
                        mybir.ActivationFunctionType.Copy)
                nc.sync.dma_start_transpose(
                    xtb_t[mt][:, h * (KT // 2):(h + 1) * (KT // 2), :], xb[:])

        # ---- matmul walk over a set of m-tiles for one n-tile
        def m_walk(ni, m0, mcnt, wrs):
            nsl = slice(ni * n_tile, (ni + 1) * n_tile)
            pss = [psum.tile([P, n_tile], f32, name="mm") for _ in range(mcnt)]
            for ki in range(KT):
                for j in range(mcnt):
                    nc.tensor.matmul(
                        pss[j][:], xtb_t[m0 + j][:, ki, :], wrs[ki][:],
                        start=(ki == 0), stop=False)
            for j in range(mcnt):
                mt = m0 + j
                nc.tensor.matmul(
                    pss[j][:], p1t[:, mt * P:(mt + 1) * P], b17[:, nsl],
                    start=False, stop=True)
                ot = outstage.tile([P, n_tile], f32, name="ot")
                nc.scalar.activation(
                    ot[:], pss[j][:], mybir.ActivationFunctionType.Copy)
                nc.scalar.dma_start(out[mt * P:(mt + 1) * P, nsl], ot[:])

        # ---- W tiles for n-tile ni: DMA + convert, tiles retained
        def load_w(ni):
            nsl = slice(ni * n_tile, (ni + 1) * n_tile)
            wrs = []
            for ki in range(KT):
                ws = wstage.tile([P, n_tile], f32, name="ws")
                nc.sync.dma_start(ws[:], W[ki * P:(ki + 1) * P, nsl])
                wr = wrpool.tile([P, n_tile], bf16, name="wr")
                nc.vector.tensor_copy(wr[:], ws[:])
                wrs.append(wr)
            return wrs

        # ---- ni = 0: quarters riding the x arrival (mi-major x order)
        wrs = load_w(0)
        for q in range(NQ):
            m0 = q * QM
            for mi in range(QM):
                load_mtile(m0 + mi)
            # p1t for the quarter's m-tiles, one borrowed bank per m-tile
            # (matmul start=True zeroes the whole PSUM bank on HW, so
            # regions of one bank cannot carry independent start flags)
            for j in range(QM):
                pt = psum.tile([P, n_tile], f32, name="mm")
                for ki in range(KT):
                    nc.tensor.matmul(
                        pt[0:r, 0:P], a_bf[:, ki, :],
                        xtb_t[m0 + j][:, ki, :],
                        start=(ki == 0), stop=(ki == KT - 1))
                nc.vector.tensor_copy(
                    p1t[0:r, (m0 + j) * P:(m0 + j + 1) * P], pt[0:r, 0:P])
            if q == NQ - 1:
                # ni=1's W DMA+converts drain while q3's k-walk frees wr
                # slots, instead of serializing after it.
                wrs_next = load_w(1)
            m_walk(0, m0, QM, wrs)

        # ---- ni >= 1: halves (8 banks), wr tiles reused across halves.
        # W for ni+1 is emitted between the halves so its DMA+convert
        # overlaps half 1 (the wr slots free as half 1's k-walk passes).
        for ni in range(1, NT):
            wrs = wrs_next
            if ni < NT - 1:
                m_walk(ni, 0, 8, wrs)
                wrs_next = load_w(ni + 1)
                m_walk(ni, 8, 8, wrs)
            else:
                # taper the final walks so the drain pipeline (copyout +
                # out DMA) empties during remaining matmuls, not after
                m_walk(ni, 0, 8, wrs)
                m_walk(ni, 8, 4, wrs)
                m_walk(ni, 12, 2, wrs)
                m_walk(ni, 14, 2, wrs)
    nc.compile()
    return nc


_CACHE = {}


def _get_nc(key, *args, **kw):
    if key not in _CACHE:
        _CACHE[key] = build_nc(*args, **kw)
    return _CACHE[key]


def kernel(x, W, bias, lora_A, lora_B, _trace=False):
    Bb, S, D = x.shape
    R = lora_A.shape[1]
    M = Bb * S
    m_core = M // NCORES
    nc = _get_nc(("v3", m_core, D, R), m_core, D, R)

    xf = np.ascontiguousarray(x.reshape(M, D), dtype=np.float32)
    W = np.ascontiguousarray(W, dtype=np.float32)
    bias = np.ascontiguousarray(bias, dtype=np.float32)
    lora_A = np.ascontiguousarray(lora_A, dtype=np.float32)
    lora_B = np.ascontiguousarray(lora_B, dtype=np.float32)

    in_maps = []
    for c in range(NCORES):
        in_maps.append({
            "x": xf[c * m_core:(c + 1) * m_core],
            "W": W, "bias": bias, "lora_A": lora_A, "lora_B": lora_B,
        })
    res = run_bass_kernel_spmd(nc, in_maps, list(range(NCORES)), trace=_trace)
    outs = [res.results[c]["out"] for c in range(NCORES)]
    full = np.concatenate(outs, axis=0).reshape(Bb, S, D).astype(x.dtype)
    if _trace:
        return full, res
    return full


# revision 8
# speedup vs baseline: 1.0381x; 1.0079x over previous
"""HQQLinearLoRA TRN2 kernel v3: out = x @ W + (x @ A) @ B + bias.

Data-parallel over tokens (16384) across 8 cores; per core
[2048, 4096] @ [4096, 4096] + rank-16 LoRA + bias.

v3 design:
- bf16 matmuls; the WHOLE core's x^T lives in SBUF ([128,16,32,128] bf16,
  128 KB/partition) so W is streamed from HBM exactly ONCE (~187 us);
  the 32 converted wr tiles of each n-tile are retained and reused by
  every m-walk of that n-tile.
- x^T via XBAR DMA-transpose (bf16, half-row per op). mi-major x order
  so ni=0 can start before all of x has arrived.
- ni=0 walks m in quarters (4 m-tiles, 4 psum banks) riding the x
  arrival; p1t (with a ones row for the fused K=17 LoRA+bias matmul)
  accumulates per m-tile on one borrowed bank (matmul start=True zeroes
  a whole PSUM bank on HW, so regions cannot share a bank). ni>=1 walks
  m in halves (8 banks), W of ni+1 prefetched between the halves.
- Engine map: Pool SEQ = x loads (SWDGE), Act = x f32->bf16 converts +
  psum copyouts + out-store DMAs (HWDGE), SP = W loads + x^T transposes
  (HWDGE), DVE = W converts + p1t copyouts. One dependency stream per
  sequencer: DMA triggers hold their SEQ while waiting, so cross-stream
  sharing causes priority inversion.
"""
import numpy as np
from contextlib import ExitStack

import concourse.bass as bass
import concourse.tile as tile
import concourse.mybir as mybir
from concourse import bacc
from concourse.bass_utils import run_bass_kernel_spmd

P = 128
NCORES = 8

B_DIM, S_DIM, D_DIM, R_DIM = 4, 4096, 4096, 16


def build_nc(m_core, d, r, n_tile=512, x_chunk=1024,
             ws_bufs=2, xs_bufs=4, ot_bufs=2, xb_bufs=2):
    KT = d // P                 # 32 k-tiles
    MT = m_core // P            # 16 m-tiles
    NT = d // n_tile            # 8 n-tiles
    XC = d // x_chunk           # 8 x chunks per m-tile row
    KC = x_chunk // P           # 4 k-tiles per x chunk
    QM = 1                      # m-tiles per ni=0 group (tracks x arrival)
    NQ = MT // QM               # 4 quarters
    f32 = mybir.dt.float32
    bf16 = mybir.dt.bfloat16
    RB = r + 1                  # fused lora+bias contraction depth

    nc = bacc.Bacc(target_bir_lowering=False)
    x = nc.declare_dram_parameter("x", [m_core, d], f32, isOutput=False)
    W = nc.declare_dram_parameter("W", [d, d], f32, isOutput=False)
    bias = nc.declare_dram_parameter("bias", [d], f32, isOutput=False)
    lora_A = nc.declare_dram_parameter("lora_A", [d, r], f32, isOutput=False)
    lora_B = nc.declare_dram_parameter("lora_B", [r, d], f32, isOutput=False)
    out = nc.declare_dram_parameter("out", [m_core, d], f32, isOutput=True)

    with tile.TileContext(nc) as tc, ExitStack() as ctx:
        const = ctx.enter_context(tc.tile_pool(name="const", bufs=1))
        stg = ctx.enter_context(tc.tile_pool(name="stg", bufs=1))
        xstage = ctx.enter_context(tc.tile_pool(name="xstage", bufs=xs_bufs))
        xbfp = ctx.enter_context(tc.tile_pool(name="xbf", bufs=xb_bufs))
        wstage = ctx.enter_context(tc.tile_pool(name="wstage", bufs=ws_bufs))
        wrpool = ctx.enter_context(tc.tile_pool(name="wrpool", bufs=KT))
        outstage = ctx.enter_context(tc.tile_pool(name="outstage", bufs=ot_bufs))
        psum = ctx.enter_context(tc.tile_pool(name="psum", bufs=8, space="PSUM"))

        # ---- consts (A staged through a borrowed ws slot: same byte size)
        a_ws = wstage.tile([P, n_tile], f32, name="ws")
        a_st = a_ws[:].rearrange("p (kt r) -> p kt r", r=r)
        nc.sync.dma_start(a_st, lora_A.rearrange("(kt p) r -> p kt r", p=P))
        a_bf = const.tile([P, KT, r], bf16, name="a_bf")
        nc.vector.tensor_copy(a_bf[:], a_st)

        b17 = const.tile([RB, d], bf16, name="b17")
        for ni in range(NT):
            sl = slice(ni * n_tile, (ni + 1) * n_tile)
            bst = stg.tile([RB, n_tile], f32, name="bst")
            nc.gpsimd.dma_start(bst[0:r, :], lora_B[:, sl])
            nc.gpsimd.dma_start(bst[r:r + 1, :], bias[sl].unsqueeze(0))
            nc.vector.tensor_copy(b17[:, sl], bst[:])

        # ones row: engine writes must start at partition 0/32/64/96, so
        # memset all RB rows to 1.0 and let the chunk copyouts overwrite
        # rows 0:16; row 16 keeps the 1.0 needed for the fused bias term.
        p1t = const.tile([RB, m_core], bf16, name="p1t")
        nc.vector.memset(p1t[:], 1.0)

        # resident x^T, one tile per m-tile (keeps dep tracking per m-tile)
        xtpool = ctx.enter_context(tc.tile_pool(name="xtpool", bufs=MT))
        xtb_t = [xtpool.tile([P, KT, P], bf16, name="xt") for _ in range(MT)]

        # ---- x chain for one m-tile: loads on Pool SWDGE, f32->bf16 on the
        # Act engine into a half-row buffer, XBAR transpose per half on SP.
        def load_mtile(mt):
            nchunk = (d // 2) // x_chunk
            for h in range(2):
                xb = xbfp.tile([P, d // 2], bf16, name="xb")
                for c in range(nchunk):
                    col = h * (d // 2) + c * x_chunk
                    xs = xstage.tile([P, x_chunk], f32, name="xs")
                    nc.gpsimd.dma_start(
                        xs[:], x[mt * P:(mt + 1) * P, col:col + x_chunk])
                    nc.scalar.activation(
                        xb[:, c * x_chunk:(c + 1) * x_chunk], xs[:],
                        mybir.ActivationFunctionType.Copy)
                nc.sync.dma_start_transpose(
                    xtb_t[mt][:, h * (KT // 2):(h + 1) * (KT // 2), :], xb[:])

        # ---- matmul walk over a set of m-tiles for one n-tile
        def m_walk(ni, m0, mcnt, wrs):
            nsl = slice(ni * n_tile, (ni + 1) * n_tile)
            pss = [psum.tile([P, n_tile], f32, name="mm") for _ in range(mcnt)]
            for ki in range(KT):
                for j in range(mcnt):
                    nc.tensor.matmul(
                        pss[j][:], xtb_t[m0 + j][:, ki, :], wrs[ki][:],
                        start=(ki == 0), stop=False)
            for j in range(mcnt):
                mt = m0 + j
                nc.tensor.matmul(
                    pss[j][:], p1t[:, mt * P:(mt + 1) * P], b17[:, nsl],
                    start=False, stop=True)
                ot = outstage.tile([P, n_tile], f32, name="ot")
                nc.scalar.activation(
                    ot[:], pss[j][:], mybir.ActivationFunctionType.Copy)
                nc.scalar.dma_start(out[mt * P:(mt + 1) * P, nsl], ot[:])

        # ---- W tiles for n-tile ni: DMA + convert, tiles retained
        def load_w(ni):
            nsl = slice(ni * n_tile, (ni + 1) * n_tile)
            wrs = []
            for ki in range(KT):
                ws = wstage.tile([P, n_tile], f32, name="ws")
                nc.sync.dma_start(ws[:], W[ki * P:(ki + 1) * P, nsl])
                wr = wrpool.tile([P, n_tile], bf16, name="wr")
                nc.vector.tensor_copy(wr[:], ws[:])
                wrs.append(wr)
            return wrs

        # ---- ni = 0: quarters riding the x arrival (mi-major x order)
        wrs = load_w(0)
        for q in range(NQ):
            m0 = q * QM
            for mi in range(QM):
                load_mtile(m0 + mi)
            # p1t for the quarter's m-tiles, one borrowed bank per m-tile
            # (matmul start=True zeroes the whole PSUM bank on HW, so
            # regions of one bank cannot carry independent start flags)
            for j in range(QM):
                pt = psum.tile([P, n_tile], f32, name="mm")
                for ki in range(KT):
                    nc.tensor.matmul(
                        pt[0:r, 0:P], a_bf[:, ki, :],
                        xtb_t[m0 + j][:, ki, :],
                        start=(ki == 0), stop=(ki == KT - 1))
                nc.vector.tensor_copy(
                    p1t[0:r, (m0 + j) * P:(m0 + j + 1) * P], pt[0:r, 0:P])
            if q == NQ - 1:
                # ni=1's W DMA+converts drain while q3's k-walk frees wr
                # slots, instead of serializing after it.
                wrs_next = load_w(1)
            m_walk(0, m0, QM, wrs)

        # ---- ni >= 1: halves (8 banks), wr tiles reused across halves.
        # W for ni+1 is emitted between the halves so its DMA+convert
        # overlaps half 1 (the wr slots free as half 1's k-walk passes).
        for ni in range(1, NT):
            wrs = wrs_next
            if ni < NT - 1:
                # quarters instead of halves: finer psum-drain boundaries
                m_walk(ni, 0, 4, wrs)
                m_walk(ni, 4, 4, wrs)
                wrs_next = load_w(ni + 1)
                m_walk(ni, 8, 4, wrs)
                m_walk(ni, 12, 4, wrs)
            else:
                # taper the final walks so the drain pipeline (copyout +
                # out DMA) empties during remaining matmuls, not after
                m_walk(ni, 0, 8, wrs)
                m_walk(ni, 8, 4, wrs)
                m_walk(ni, 12, 2, wrs)
                m_walk(ni, 14, 2, wrs)
    nc.compile()
    return nc


_CACHE = {}


def _get_nc(key, *args, **kw):
    if key not in _CACHE:
        _CACHE[key] = build_nc(*args, **kw)
    return _CACHE[key]


def kernel(x, W, bias, lora_A, lora_B, _trace=False):
    Bb, S, D = x.shape
    R = lora_A.shape[1]
    M = Bb * S
    m_core = M // NCORES
    nc = _get_nc(("v3", m_core, D, R), m_core, D, R)

    xf = np.ascontiguousarray(x.reshape(M, D), dtype=np.float32)
    W = np.ascontiguousarray(W, dtype=np.float32)
    bias = np.ascontiguousarray(bias, dtype=np.float32)
    lora_A = np.ascontiguousarray(lora_A, dtype=np.float32)
    lora_B = np.ascontiguousarray(lora_B, dtype=np.float32)

    in_maps = []
    for c in range(NCORES):
        in_maps.append({
            "x": xf[c * m_core:(c + 1) * m_core],
            "W": W, "bias": bias, "lora_A": lora_A, "lora_B": lora_B,
        })
    res = run_bass_kernel_spmd(nc, in_maps, list(range(NCORES)), trace=_trace)
    outs = [res.results[c]["out"] for c in range(NCORES)]
    full = np.concatenate(outs, axis=0).reshape(Bb, S, D).astype(x.dtype)
    if _trace:
        return full, res
    return full
